# revision 9
# baseline (speedup 1.0000x reference)
"""Bahdanau (additive) attention Trainium2 kernel.

Full-input contract: kernel(**inputs) takes the unsharded inputs
(query [16,128,256], value [16,256,256], mask [16,256], W1 [256,256],
W2 [256,256], scale [256]) and returns (context, attn_weights), both
[16,128,256] float32, matching the jax reference.

Sharding: data-parallel over batch -> 8 NeuronCores x 2 batches each.

Per-core algorithm (per batch b; t=128 query rows, s=256 kv rows, u=256):
  1. q = query @ W1  [t,u],  k = value @ W2  [s,u]   (PE, fp32)
  2. outer-sums x[u, t, s] = q[t,u] + k[s,u] built by tiny selector
     matmuls: lhsT = staged [q-rows(32); k-rows(16)] fp16, rhs = constant
     0/1 selector [48, 512] -> PSUM chunks [128u, 512 pairs]
  3. tanh on ScalarE in large fused passes PSUM->SBUF fp16, laid out
     [u, ub, t*256+s] so it can feed the contraction directly
  4. scores[t,s] = sum_u scale_u * tanh(...): M=1 matmuls (lhsT=scale
     column) across 4 PE column groups; small DMAs redistribute the
     PSUM rows into a [t, s] SBUF tile
  5. masked softmax over s (no max-subtraction needed: |scores| <= ~13),
     row sums via activation accum_out
  6. context = attn @ value (PE, fp16 inputs, fp32 accum)
"""

import sys

if "/opt/trn_rl_repo" not in sys.path:
    sys.path.insert(0, "/opt/trn_rl_repo")

from contextlib import ExitStack

import numpy as np

import concourse.bacc as bacc
import concourse.bass as bass
import concourse.tile as tile
from concourse import mybir
from concourse.bass_utils import run_bass_kernel_spmd

F32 = mybir.dt.float32
F16 = mybir.dt.float16
U8 = mybir.dt.uint8
AF = mybir.ActivationFunctionType

N_CORES = 8
B = 2          # batches per core
T = 128        # query rows
S = 256        # kv rows
D = 256        # d_model
U = 256        # units
TC = 32        # t-rows per outer-sum chunk
SC = 16        # s-rows per outer-sum chunk
NEG = -30000.0


def _selectors() -> np.ndarray:
    """[2, TC+32, TC*SC] 0/1 matrices.

    Staged lhsT holds 32 q-rows (partitions 0..31) and a 32-aligned block of
    32 k-rows (partitions 32..63).  Variant h selects k rows 32+16h..47+16h:
    out[u, t'*SC+s'] = q[t',u] + k[16h+s',u].
    """
    sel = np.zeros((2, TC + 32, TC * SC), dtype=np.float16)
    for h in range(2):
        for tp in range(TC):
            sel[h, tp, tp * SC : (tp + 1) * SC] = 1.0
        for sp in range(SC):
            sel[h, TC + 16 * h + sp, sp::SC] = 1.0
    return sel


def build_bass() -> bass.Bass:
    nc = bacc.Bacc("TRN2", target_bir_lowering=False, debug=False)

    q_in = nc.dram_tensor("query", [B, T, D], F32, kind="ExternalInput")
    v_in = nc.dram_tensor("value", [B, S, D], F32, kind="ExternalInput")
    m_in = nc.dram_tensor("mask", [B, S], U8, kind="ExternalInput")
    w1_in = nc.dram_tensor("W1", [D, U], F32, kind="ExternalInput")
    w2_in = nc.dram_tensor("W2", [D, U], F32, kind="ExternalInput")
    sc_in = nc.dram_tensor("scale", [U], F32, kind="ExternalInput")
    ctx_out = nc.dram_tensor("context", [B, T, D], F32, kind="ExternalOutput")
    attn_out = nc.dram_tensor("attn", [B, T, S], F32, kind="ExternalOutput")

    sel_d = nc.inline_tensor(_selectors(), "sel_const")
    id32_d = nc.inline_tensor(np.eye(128, dtype=np.float32), "id32_const")

    with tile.TileContext(nc) as tc, ExitStack() as ctx:
        singles = ctx.enter_context(tc.tile_pool(name="singles", bufs=1))
        perb = ctx.enter_context(tc.tile_pool(name="perb", bufs=2))
        stage = ctx.enter_context(tc.tile_pool(name="stage", bufs=2))
        tanh_pool = ctx.enter_context(tc.tile_pool(name="tanh", bufs=2))
        p_outer = ctx.enter_context(tc.tile_pool(name="p_outer", bufs=3, space="PSUM"))
        p_misc = ctx.enter_context(tc.tile_pool(name="p_misc", bufs=2, space="PSUM"))

        # ---- constants into SBUF
        sel_sb = singles.tile([TC + 32, 2, TC * SC], F16)
        nc.sync.dma_start(
            out=sel_sb, in_=sel_d.rearrange("h p n -> p h n")
        )
        id32 = singles.tile([128, 128], F32)
        nc.sync.dma_start(out=id32, in_=id32_d[:, :])
        w1_sb = singles.tile([128, 2, U], F32)
        nc.sync.dma_start(out=w1_sb, in_=w1_in.rearrange("(a p) u -> p a u", a=2))
        w2_sb = singles.tile([128, 2, U], F32)
        nc.sync.dma_start(out=w2_sb, in_=w2_in.rearrange("(a p) u -> p a u", a=2))
        scale_f = singles.tile([128, 2], F32)
        nc.sync.dma_start(out=scale_f, in_=sc_in.rearrange("(a p) -> p a", a=2))
        scale16 = singles.tile([128, 2], F16)
        nc.vector.tensor_copy(out=scale16, in_=scale_f)

        for b in range(B):
            # ---------------- load inputs for this batch
            query_sb = perb.tile([T, D], F32, tag="query")
            nc.sync.dma_start(out=query_sb, in_=q_in[b])
            value_sb = perb.tile([128, 2, D], F32, tag="value")
            nc.sync.dma_start(
                out=value_sb, in_=v_in[b].rearrange("(a p) d -> p a d", a=2)
            )
            mask_row = m_in[b, :]
            mask_u8 = perb.tile([T, S], U8, tag="mask_u8")
            nc.sync.dma_start(
                out=mask_u8,
                in_=bass.AP(
                    tensor=mask_row.tensor,
                    offset=mask_row.offset,
                    ap=[[0, T]] + list(mask_row.ap),
                ),
            )
            mask_f = perb.tile([T, S], F32, tag="mask_f")
            nc.vector.tensor_copy(out=mask_f, in_=mask_u8)
            maskb = perb.tile([T, S], F32, tag="maskb")
            # (m - 1) * 30000: 0 where mask on, -30000 where off
            nc.vector.tensor_scalar(
                out=maskb,
                in0=mask_f,
                scalar1=-NEG,
                scalar2=NEG,
                op0=mybir.AluOpType.mult,
                op1=mybir.AluOpType.add,
            )

            # ---------------- qT / vT via PE transposes
            qT = perb.tile([128, 2, T], F32, tag="qT")
            for j in range(2):
                pt = p_misc.tile([128, 128], F32, tag="misc")
                nc.tensor.transpose(pt, query_sb[:, j * 128 : (j + 1) * 128], id32)
                nc.vector.tensor_copy(out=qT[:, j, :], in_=pt)
            vT = perb.tile([128, 2, S], F32, tag="vT")
            for sblk in range(2):
                for j in range(2):
                    pt = p_misc.tile([128, 128], F32, tag="misc")
                    nc.tensor.transpose(
                        pt, value_sb[:, sblk, j * 128 : (j + 1) * 128], id32
                    )
                    nc.vector.tensor_copy(
                        out=vT[:, j, sblk * 128 : (sblk + 1) * 128], in_=pt
                    )

            # ---------------- q = query @ W1 -> fp16 [t, u]
            qp = p_misc.tile([T, U], F32, tag="misc")
            for j in range(2):
                nc.tensor.matmul(
                    qp, lhsT=qT[:, j, :], rhs=w1_sb[:, j, :],
                    start=(j == 0), stop=(j == 1),
                )
            q16 = perb.tile([T, U], F16, tag="q16")
            nc.vector.tensor_copy(out=q16, in_=qp)

            # ---------------- k = value @ W2 -> fp16 [s(2 blk), u]
            k16 = perb.tile([128, 2, U], F16, tag="k16")
            for sblk in range(2):
                kp = p_misc.tile([128, U], F32, tag="misc")
                for j in range(2):
                    nc.tensor.matmul(
                        kp,
                        lhsT=vT[:, j, sblk * 128 : (sblk + 1) * 128],
                        rhs=w2_sb[:, j, :],
                        start=(j == 0), stop=(j == 1),
                    )
                nc.vector.tensor_copy(out=k16[:, sblk, :], in_=kp)

            scores_sb = perb.tile([T, S], F32, tag="scores")

            # ---------------- big loop over t-groups
            for tg in range(4):
                tanh_t = tanh_pool.tile([128, 2, TC * S], F16, tag="tanh")
                stA = stage.tile([TC + 32, U], F16, tag="stgA")
                stB = stage.tile([TC + 32, U], F16, tag="stgB")
                nc.vector.tensor_copy(
                    out=stA[0:TC, :], in_=q16[tg * TC : (tg + 1) * TC, :]
                )
                nc.vector.tensor_copy(
                    out=stB[0:TC, :], in_=q16[tg * TC : (tg + 1) * TC, :]
                )
                for sp in range(8):
                    st = stA if sp % 2 == 0 else stB
                    sblk, row32 = divmod(sp, 4)
                    nc.vector.tensor_copy(
                        out=st[TC : TC + 32, :],
                        in_=k16[row32 * 32 : (row32 + 1) * 32, sblk, :],
                    )
                    for half in range(2):
                        sg = 2 * sp + half
                        pot = p_outer.tile([128, 2, TC * SC], F32, tag="outer")
                        for ub in range(2):
                            nc.tensor.matmul(
                                pot[:, ub, :],
                                lhsT=st[:, ub * 128 : (ub + 1) * 128],
                                rhs=sel_sb[:, half, :],
                                start=True, stop=True,
                            )
                        # tanh: psum [128, (ub, t', s')]
                        #   -> tanh_t[:, ub, t'*256 + sg*16 + s']
                        dst = tanh_t.rearrange("p a (t s) -> p a t s", s=S)[
                            :, :, :, sg * SC : (sg + 1) * SC
                        ]
                        src = pot.rearrange("p a (t s) -> p a t s", t=TC)
                        nc.scalar.activation(out=dst, in_=src, func=AF.Tanh)

                # contraction: scores[t, :] = sum_u scale_u * tanh
                for qr in range(TC // 4):
                    scq = p_misc.tile([128, S], F32, tag="misc")
                    for i in range(4):
                        tloc = qr * 4 + i
                        for ub in range(2):
                            nc.tensor.matmul(
                                scq[32 * i : 32 * i + 1, :],
                                lhsT=scale16[:, ub : ub + 1],
                                rhs=tanh_t[:, ub, tloc * S : (tloc + 1) * S],
                                start=(ub == 0), stop=(ub == 1),
                                tile_position=(0, 32 * i),
                            )
                    scstg = perb.tile([128, S], F32, tag="scstg")
                    nc.vector.tensor_copy(out=scstg, in_=scq)
                    t0 = tg * TC + qr * 4
                    nc.sync.dma_start(
                        out=scores_sb[t0 : t0 + 4, :],
                        in_=scstg.rearrange("(a r) s -> a r s", r=32)[:, 0, :],
                    )

            # ---------------- masked softmax over s
            nc.vector.tensor_add(out=scores_sb, in0=scores_sb, in1=maskb)
            exp_sb = perb.tile([T, S], F32, tag="exp")
            sums = perb.tile([T, 1], F32, tag="sums")
            nc.scalar.activation(
                out=exp_sb, in_=scores_sb, func=AF.Exp, accum_out=sums
            )
            inv = perb.tile([T, 1], F32, tag="inv")
            nc.vector.reciprocal(out=inv, in_=sums)
            attn_f = perb.tile([T, S], F32, tag="attn_f")
            nc.vector.tensor_scalar_mul(out=attn_f, in0=exp_sb, scalar1=inv)
            nc.sync.dma_start(out=attn_out[b], in_=attn_f)

            # ---------------- context = attn @ value
            attnT = perb.tile([128, 2, T], F16, tag="attnT")
            for sblk in range(2):
                pt = p_misc.tile([128, 128], F32, tag="misc")
                nc.tensor.transpose(pt, attn_f[:, sblk * 128 : (sblk + 1) * 128], id32)
                nc.vector.tensor_copy(out=attnT[:, sblk, :], in_=pt)
            v16 = perb.tile([128, 2, D], F16, tag="v16")
            nc.vector.tensor_copy(out=v16, in_=value_sb)
            ctxp = p_misc.tile([T, D], F32, tag="misc")
            for sblk in range(2):
                nc.tensor.matmul(
                    ctxp,
                    lhsT=attnT[:, sblk, :],
                    rhs=v16[:, sblk, :],
                    start=(sblk == 0), stop=(sblk == 1),
                )
            ctx_f = perb.tile([T, D], F32, tag="ctx_f")
            nc.vector.tensor_copy(out=ctx_f, in_=ctxp)
            nc.sync.dma_start(out=ctx_out[b], in_=ctx_f)

    nc.compile()
    return nc


_BUILT: bass.Bass | None = None


def _get_built() -> bass.Bass:
    global _BUILT
    if _BUILT is None:
        _BUILT = build_bass()
    return _BUILT


def make_in_maps(query, value, mask, W1, W2, scale):
    q = np.ascontiguousarray(np.asarray(query, dtype=np.float32))
    v = np.ascontiguousarray(np.asarray(value, dtype=np.float32))
    m = np.ascontiguousarray(np.asarray(mask).astype(np.uint8))
    w1 = np.ascontiguousarray(np.asarray(W1, dtype=np.float32))
    w2 = np.ascontiguousarray(np.asarray(W2, dtype=np.float32))
    sc = np.ascontiguousarray(np.asarray(scale, dtype=np.float32))
    in_maps = []
    for c in range(N_CORES):
        sl = slice(B * c, B * (c + 1))
        in_maps.append(
            {
                "query": np.ascontiguousarray(q[sl]),
                "value": np.ascontiguousarray(v[sl]),
                "mask": np.ascontiguousarray(m[sl]),
                "W1": w1,
                "W2": w2,
                "scale": sc,
            }
        )
    return in_maps


def run(query, value, mask, W1, W2, scale, trace=False, **trace_kwargs):
    nc = _get_built()
    in_maps = make_in_maps(query, value, mask, W1, W2, scale)
    res = run_bass_kernel_spmd(
        nc, in_maps, core_ids=list(range(N_CORES)), trace=trace, **trace_kwargs
    )
    context = np.concatenate([r["context"] for r in res.results], axis=0)
    attn = np.concatenate([r["attn"] for r in res.results], axis=0)
    return (context, attn), res


def kernel(query, value, mask, W1, W2, scale):
    (context, attn), _ = run(query, value, mask, W1, W2, scale, trace=False)
    return context, attn


if __name__ == "__main__":
    build_bass()
    print("build OK")
